# revision 52
# baseline (speedup 1.0000x reference)
"""Causal self-attention (GPT-2 small block shape: B=4, T=2048, C=768, H=12, D=64)
on 8 TRN2 NeuronCores.

Sharding: core i handles batch b = i//2 and head-half = i%2 (6 heads each).
No cross-core collectives; the two half-head partial output projections per
batch are summed on the host during unshard (row-parallel c_proj).

Optimizations over the first working version (228us -> 206us):
  - S matmuls for the two heads of a pair are issued back-to-back as
    64-contraction row-tiles (rows 0:64 / 64:128 of the PE array) so they
    stream CONCURRENTLY -> S-gen PE cost halves.
  - One ScalarE exp per k-chunk covers both heads ([128,1024] f32 PSUM
    tile), with a strided AP that skips the causally-dead leading columns
    of diagonal chunks; exp table preloaded at kernel start.
  - Global software pipeline: AV of chunk j is emitted during chunk j+1;
    QKV/V/proj matmuls are scheduled as PE fill into the ACT-bound late
    rounds; av-bank evacuation is decoupled from the reciprocal/broadcast/
    multiply normalization tail (which drains one-per-chunk in later slack)
    so the next head-pair's AV is never WAR-blocked on the norm chain.
  - V/proj biases folded in as K=1 ones-outer-product matmuls; reciprocal
    runs on a partition-0 copy (the custom DVE recip uop cannot read
    across partitions - plain COPY can).
  - Input DMAs split per-use across sync/scalar/gpsimd queues in
    first-use order (first matmul at ~13us vs ~23us); output stored as
    bf16 (host upcasts + sums); keep-warm matmuls bridge the tail norm
    chain so the final projections don't run HAM-throttled.
"""

import sys

if "/opt/trn_rl_repo" not in sys.path:
    sys.path.insert(0, "/opt/trn_rl_repo")

import numpy as np
import ml_dtypes

import concourse.bass as bass  # noqa: F401
import concourse.mybir as mybir
from concourse import bacc
from concourse.tile import TileContext
from concourse.bass_utils import run_bass_kernel_spmd

BF16 = ml_dtypes.bfloat16

B, T, C = 4, 2048, 768
H, D = 12, 64
NH = 6  # heads per core
P = 128
TC = T // P  # 16 t-chunks of 128
QC = T // 512  # 4 q-blocks of 512
CCH = C // P  # 6 contraction chunks

DT = mybir.dt.bfloat16
F32 = mybir.dt.float32


def build_nc():
    nc = bacc.Bacc()

    xt_d = nc.declare_dram_parameter("xt", [P, T // 512, CCH, 512], DT, isOutput=False)
    # fc-major so per-fc chunks are a single contiguous DMA
    wqk_d = nc.declare_dram_parameter("wqk", [P, 2 * NH * D // P, CCH, P], DT, isOutput=False)
    bqk_d = nc.declare_dram_parameter("bqk", [P, 2 * NH * D // P], F32, isOutput=False)
    wv_d = nc.declare_dram_parameter("wv", [P, CCH, NH * D], DT, isOutput=False)
    bv_d = nc.declare_dram_parameter("bv", [1, NH * D], DT, isOutput=False)
    wp_d = nc.declare_dram_parameter("wp", [P, NH * D // P, C], DT, isOutput=False)
    bp_d = nc.declare_dram_parameter("bp", [1, C], DT, isOutput=False)
    mask_d = nc.declare_dram_parameter("mask", [P, 2 * P], DT, isOutput=False)
    out_d = nc.declare_dram_parameter("out", [T, C], DT, isOutput=True)

    NFC = 2 * NH * D // P  # 6 q/k feature chunks

    with TileContext(nc) as tc:
        with (
            tc.tile_pool(name="consts", bufs=1) as consts,
            tc.tile_pool(name="work", bufs=3) as work,
            tc.tile_pool(name="outp", bufs=3) as outp,
            tc.tile_pool(name="ps_s", bufs=2, space="PSUM") as ps_s,
            tc.tile_pool(name="ps_qkv", bufs=2, space="PSUM") as ps_qkv,
            tc.tile_pool(name="ps_av", bufs=2, space="PSUM") as ps_av,
        ):
            # ---- input DMAs, ordered by first use ----
            # xt is quarter-major [P, 4, CCH, 512]; each quarter's halves
            # (cc 0:3 / 3:6) stream on sync/scalar in parallel.
            bqk_sb = consts.tile([P, NFC], F32)
            nc.gpsimd.dma_start(bqk_sb[:], bqk_d[:])
            xt_sb = consts.tile([P, QC, CCH, 512], DT)
            wqk_sb = consts.tile([P, NFC, CCH, P], DT)
            nc.sync.dma_start(xt_sb[:, 0, 0:3], xt_d[:, 0, 0:3])
            nc.scalar.dma_start(xt_sb[:, 0, 3:6], xt_d[:, 0, 3:6])
            nc.sync.dma_start(wqk_sb[:, 0], wqk_d[:, 0])
            nc.scalar.dma_start(wqk_sb[:, 3], wqk_d[:, 3])
            nc.sync.dma_start(wqk_sb[:, 1], wqk_d[:, 1])
            nc.scalar.dma_start(wqk_sb[:, 4], wqk_d[:, 4])
            nc.sync.dma_start(wqk_sb[:, 2], wqk_d[:, 2])
            nc.scalar.dma_start(wqk_sb[:, 5], wqk_d[:, 5])
            mask2_sb = consts.tile([P, 2 * P], DT)
            nc.gpsimd.dma_start(mask2_sb[:], mask_d[:])
            bv_sb = consts.tile([1, NH * D], DT)
            nc.gpsimd.dma_start(bv_sb[:], bv_d[:])
            wv_sb = consts.tile([P, CCH, NH * D], DT)
            nc.sync.dma_start(wv_sb[:], wv_d[:])
            for q in range(1, QC):
                nc.sync.dma_start(xt_sb[:, q, 0:3], xt_d[:, q, 0:3])
                nc.scalar.dma_start(xt_sb[:, q, 3:6], xt_d[:, q, 3:6])
            bp_sb = consts.tile([1, C], DT)
            nc.gpsimd.dma_start(bp_sb[:], bp_d[:])
            wp_sb = consts.tile([P, NH * D // P, C], DT)
            nc.gpsimd.dma_start(wp_sb[:], wp_d[:])

            # ones row for K=1 bias-broadcast matmuls (no DMA dependency)
            ones_row = consts.tile([1, P], DT)
            nc.gpsimd.memset(ones_row[:], 1.0)
            bpb = consts.tile([P, C], DT)
            nc.gpsimd.partition_broadcast(bpb[:], bp_sb[:])
            # preload the exp table set while input DMAs stream
            warm = consts.tile([1, 16], F32)
            nc.gpsimd.memset(warm[:], 0.0)
            warm2 = consts.tile([1, 16], DT)
            nc.scalar.activation(warm2[:], warm[:], mybir.ActivationFunctionType.Exp)

            # Q^T/K^T head-pair tiles [128, T]: head 2p in partitions 0:64,
            # head 2p+1 in partitions 64:128
            qtp = [consts.tile([P, T], DT, name=f"qtp{p}", tag=f"qtp{p}") for p in range(NH // 2)]
            ktp = [consts.tile([P, T], DT, name=f"ktp{p}", tag=f"ktp{p}") for p in range(NH // 2)]
            # V per t-chunk, heads side by side with a ones column: [128, 6, 65]
            vt = [consts.tile([P, NH, D + 1], DT, name=f"vt{t}", tag=f"vt{t}") for t in range(TC)]
            for t in range(TC):
                nc.gpsimd.memset(vt[t][:, :, D : D + 1], 1.0)
            # y^T per head-pair [128, T] bf16
            yt = [consts.tile([P, T], DT, name=f"yt{p}", tag=f"yt{p}") for p in range(NH // 2)]

            # ---- QKV / proj helpers ----
            def qk_tile(fc, qcb):
                pq = ps_qkv.tile([P, 512], F32, tag="qkv", name="pq")
                for cc in range(CCH):
                    nc.tensor.matmul(
                        pq[:],
                        wqk_sb[:, fc, cc, :],
                        xt_sb[:, qcb, cc, :],
                        start=(cc == 0),
                        stop=(cc == CCH - 1),
                    )
                dst = qtp[fc] if fc < 3 else ktp[fc - 3]
                nc.vector.tensor_scalar_add(
                    dst[:, qcb * 512 : (qcb + 1) * 512],
                    pq[:],
                    bqk_sb[:, fc : fc + 1],
                )

            def v_chunk(t):
                pv = ps_qkv.tile([P, NH * D], F32, tag="qkv", name="pv")
                for cc in range(CCH):
                    nc.tensor.matmul(
                        pv[:],
                        xt_sb[:, t // 4, cc, (t % 4) * P : (t % 4 + 1) * P],
                        wv_sb[:, cc, :],
                        start=(cc == 0),
                        stop=False,
                    )
                # bias via K=1 outer product: ones[1,128].T @ bv[1,384]
                nc.tensor.matmul(
                    pv[:], ones_row[:], bv_sb[:], start=False, stop=True
                )
                nc.vector.tensor_copy(
                    vt[t][:, :, 0:D], pv[:].rearrange("p (h d) -> p h d", d=D)
                )

            def proj(t, pool=None, fold_bias=False):
                pool = pool or ps_qkv
                tg = "qkv" if pool is ps_qkv else "av"
                ppa = pool.tile([P, 512], F32, tag=tg, name="ppa")
                ppb = pool.tile([P, C - 512], F32, tag=tg, name="ppb")
                for pp, c0, c1 in ((ppa, 0, 512), (ppb, 512, C)):
                    for cp in range(NH * D // P):
                        nc.tensor.matmul(
                            pp[:, 0 : c1 - c0],
                            yt[cp][:, t * P : (t + 1) * P],
                            wp_sb[:, cp, c0:c1],
                            start=(cp == 0),
                            stop=(cp == NH * D // P - 1) and not fold_bias,
                        )
                    if fold_bias:
                        nc.tensor.matmul(
                            pp[:, 0 : c1 - c0],
                            ones_row[:],
                            bp_sb[:, c0:c1],
                            start=False,
                            stop=True,
                        )
                stg = outp.tile([P, C], DT)
                if fold_bias:
                    nc.vector.tensor_copy(stg[:, 0:512], ppa[:])
                    nc.vector.tensor_copy(stg[:, 512:C], ppb[:])
                else:
                    nc.vector.tensor_add(stg[:, 0:512], ppa[:], bpb[:, 0:512])
                    nc.vector.tensor_add(stg[:, 512:C], ppb[:], bpb[:, 512:C])
                nc.sync.dma_start(out_d[t * P : (t + 1) * P, :], stg[:])

            # ---- attention pieces ----
            # per (hp, qc): loop over groups g of 2 k-chunks; S for both heads
            # of the pair as concurrent row-tiles into one bf16 psum tile;
            # one exp per group; AV emitted one group behind (pipelined).

            av_tiles = {}

            def emit_s_group(hp, qc, j):
                # S for both heads of the pair as concurrent 64-row tiles:
                # head 0 -> [0:512] (bank a), head 1 -> [512:1024] (bank b)
                m = max(0, (j - 4 * qc) * P)
                sps = ps_s.tile([P, 1024], F32, tag="s", name="sps")
                for hi in (0, 1):
                    b0 = 64 * hi
                    nc.tensor.matmul(
                        sps[:, hi * 512 + m : (hi + 1) * 512],
                        ktp[hp][b0 : b0 + 64, j * P : (j + 1) * P],
                        qtp[hp][b0 : b0 + 64, qc * 512 + m : (qc + 1) * 512],
                        start=True,
                        stop=True,
                    )
                sexp = work.tile([P, 1024], DT, tag="sexp", bufs=6, name="sexp")
                if m:
                    # skip the causally-dead leading m columns of each half
                    nc.scalar.activation(
                        sexp[:].rearrange("p (h q) -> p h q", h=2)[:, :, m:],
                        sps[:].rearrange("p (h q) -> p h q", h=2)[:, :, m:],
                        mybir.ActivationFunctionType.Exp,
                    )
                else:
                    nc.scalar.activation(
                        sexp[:], sps[:], mybir.ActivationFunctionType.Exp
                    )
                if j - 4 * qc >= 0:  # diagonal block: mask upper triangle
                    # one strided TT covers both heads' diagonal squares
                    view = sexp[:].rearrange("p (h q) -> p h q", h=2)[:, :, m : m + P]
                    nc.vector.tensor_mul(
                        view, view, mask2_sb[:].rearrange("p (h q) -> p h q", h=2)
                    )
                return sexp

            def emit_av_group(hp, qc, j, sexp):
                nj = 4 * (qc + 1)
                m = max(0, (j - 4 * qc) * P)
                for hi in (0, 1):
                    av = av_tiles[(hp, qc, hi)]
                    nc.tensor.matmul(
                        av[:, m:512],
                        vt[j][:, 2 * hp + hi, :],
                        sexp[:, hi * 512 + m : (hi + 1) * 512],
                        start=(j == 0),
                        stop=(j == nj - 1),
                        skip_group_check=True,
                    )

            # Normalization is split: a fast PSUM->SBUF evacuation (frees the
            # av bank ~2us earlier so the next head-pair's AV isn't WAR-
            # blocked), then a deferred recip/broadcast/multiply chain that
            # runs in later slack.
            def emit_evac(hp, qc, hi):
                av = av_tiles.pop((hp, qc, hi))
                avf = work.tile([65, 512], F32, tag="avf", bufs=6, name="avf")
                nc.vector.tensor_copy(avf[:], av[:])
                return avf

            last_inv = [None]

            def emit_norm_tail(hp, qc, hi, avf):
                b0 = 64 * hi
                # partition-shift copy (64 -> 0): reciprocal's custom uop
                # cannot read across partitions, plain COPY can
                sums = work.tile([1, 512], F32, tag="sums", name="sums")
                nc.vector.tensor_copy(sums[:], avf[64:65, :])
                inv = work.tile([1, 512], F32, tag="inv", name="inv")
                nc.vector.reciprocal_approx_fast(inv[:], sums[:])
                invb = work.tile([64, 512], F32, tag="invb", name="invb")
                nc.gpsimd.partition_broadcast(invb[:], inv[:])
                nc.vector.tensor_mul(
                    yt[hp][b0 : b0 + 64, qc * 512 : (qc + 1) * 512],
                    avf[0:64, :],
                    invb[:],
                )
                last_inv[0] = inv

            # ---- prologue: only head-pair 0's QKV; the rest of round-0
            # QKV/V is force-woven into the qc0 attention stream so exp
            # starts ~4us earlier (positions preserve write-before-read) ----
            qk_tile(0, 0)
            qk_tile(3, 0)

            # ---- global pipeline over rounds ----
            # Per head-pair: AV of group g is emitted during group g+1 so the
            # ScalarE exp hides behind PE work; the last AV + normalization
            # are emitted before the next head-pair's masks enter the DVE
            # queue (avoids head-of-line blocking of the norm chain).
            deferred = []  # (hp, qc, hi, ytu, sums) norm tails awaiting slack

            def drain_norms(limit=None):
                n = 0
                while deferred and (limit is None or n < limit):
                    emit_norm_tail(*deferred.pop(0))
                    n += 1

            # proj(t) is scheduled late so the ACT-bound rounds 2-3 get the
            # most PE fill work; round 3 holds 2 projs back for the tail.
            proj_rounds = {1: [0, 1], 2: [2, 3, 4, 5], 3: [6, 7, 8, 9, 10, 11]}
            for qc in range(QC):
                nj = 4 * (qc + 1)
                fill = []
                if qc < QC - 1:
                    for fc in (0, 3, 1, 4, 2, 5):
                        fill.append(lambda fc=fc: qk_tile(fc, qc + 1))
                    for t in range(4 * (qc + 1), 4 * (qc + 2)):
                        fill.append(lambda t=t: v_chunk(t))
                for t in proj_rounds.get(qc, []):
                    fill.append(lambda t=t: proj(t))
                reserve = 2 if qc == QC - 1 else 0
                pace = (len(fill) - reserve) / (3 * nj)
                acc = 0.0
                fi = 0
                for hp in range(3):
                    if qc == 0 and hp > 0:
                        # this head-pair's QKV tiles must precede its S-matmuls
                        qk_tile(hp, 0)
                        qk_tile(hp + 3, 0)
                    for hi in (0, 1):
                        av_tiles[(hp, qc, hi)] = ps_av.tile(
                            [65, 512], F32, tag="av", name="av"
                        )
                    prev = None
                    for j in range(nj):
                        sexp = emit_s_group(hp, qc, j)
                        if qc == 0 and hp == 0:
                            # v_chunk(j) before AV(j) consumes vt[j]
                            v_chunk(j)
                        if prev is not None:
                            emit_av_group(hp, qc, j - 1, prev)
                            drain_norms(limit=1)
                        prev = sexp
                        acc += pace
                        while acc >= 1.0 and fi < len(fill):
                            fill[fi]()
                            fi += 1
                            acc -= 1.0
                    if fi < len(fill) - reserve:  # cover this pair's final exp
                        fill[fi]()
                        fi += 1
                        acc -= 1.0
                    emit_av_group(hp, qc, nj - 1, prev)
                    for hi in (0, 1):
                        avf = emit_evac(hp, qc, hi)
                        deferred.append((hp, qc, hi, avf))
                if qc == QC - 1:
                    # start the final norm chains, then cover them with the
                    # reserved projs + keep-warm matmuls so the PE doesn't
                    # HAM-throttle before the last projections
                    drain_norms()
                while fi < len(fill):
                    fill[fi]()
                    fi += 1
            for w in range(16):
                wps = ps_s.tile([P, 1024], F32, tag="s", name="wps")
                nc.tensor.matmul(
                    wps[:, 0:512], ones_row[:], bp_sb[:, 0:512], start=True, stop=True
                )
            for t in range(4 * (QC - 1), 4 * QC):
                proj(t, pool=(ps_qkv if t % 2 == 0 else ps_av), fold_bias=True)

    nc.finalize()
    return nc


def shard_inputs(x, w_attn, b_attn, w_proj, b_proj):
    """Host-side prep: slice per core, transpose x, cast to bf16."""
    scale = 1.0 / np.sqrt(D)
    tril = np.tril(np.ones((P, P), np.float32))
    # mask[k_local, q_local] = 1 where k <= q; duplicated for the two heads
    mask = np.concatenate([tril.T, tril.T], axis=1).astype(BF16)
    NFC = 2 * NH * D // P
    in_maps = []
    for core in range(8):
        b, half = divmod(core, 2)
        h0 = half * NH
        cq = slice(h0 * D, (h0 + NH) * D)
        ck = slice(C + h0 * D, C + (h0 + NH) * D)
        cv = slice(2 * C + h0 * D, 2 * C + (h0 + NH) * D)
        wq = (w_attn[:, cq] * scale).astype(BF16)
        wk = w_attn[:, ck].astype(BF16)
        wqk = np.concatenate([wq, wk], axis=1)  # [C, 768]
        bqk = np.concatenate([(b_attn[cq] * scale), b_attn[ck]], axis=0).astype(
            np.float32
        )
        bqk_col = np.ascontiguousarray(bqk.reshape(NFC, P).T)
        wv = w_attn[:, cv].astype(BF16)
        bv = b_attn[cv].astype(BF16)[None, :]
        wp = w_proj[h0 * D : (h0 + NH) * D, :].astype(BF16)
        bp = (b_proj if half == 0 else np.zeros_like(b_proj)).astype(BF16)[None, :]
        xt = np.ascontiguousarray(x[b].T).astype(BF16)  # [C, T]
        # wqk fc-major: [P, fc, cc, 128] with wqk[cc*128+p, fc*128+k]
        wqk_dev = np.ascontiguousarray(
            wqk.reshape(CCH, P, NFC, P).transpose(1, 2, 0, 3)
        )
        in_maps.append(
            {
                # quarter-major: [P, q, cc, 512]
                "xt": np.ascontiguousarray(
                    xt.reshape(CCH, P, T // 512, 512).transpose(1, 2, 0, 3)
                ),
                "wqk": wqk_dev,
                "bqk": bqk_col,
                "wv": np.ascontiguousarray(
                    wv.reshape(CCH, P, NH * D).transpose(1, 0, 2)
                ),
                "bv": bv,
                "wp": np.ascontiguousarray(
                    wp.reshape(NH * D // P, P, C).transpose(1, 0, 2)
                ),
                "bp": bp,
                "mask": mask,
            }
        )
    return in_maps


_NC = None


def _get_nc():
    global _NC
    if _NC is None:
        _NC = build_nc()
    return _NC


def run_sharded(in_maps, trace=False, **kw):
    nc = _get_nc()
    return run_bass_kernel_spmd(nc, in_maps, core_ids=list(range(8)), trace=trace, **kw)


def gather(results):
    out = np.zeros((B, T, C), np.float32)
    for core in range(8):
        b = core // 2
        out[b] += results[core]["out"].astype(np.float32)
    return out


def kernel(x, w_attn, b_attn, w_proj, b_proj):
    x = np.asarray(x, np.float32)
    w_attn = np.asarray(w_attn, np.float32)
    b_attn = np.asarray(b_attn, np.float32)
    w_proj = np.asarray(w_proj, np.float32)
    b_proj = np.asarray(b_proj, np.float32)
    in_maps = shard_inputs(x, w_attn, b_attn, w_proj, b_proj)
    res = run_sharded(in_maps, trace=False)
    return gather(res.results)


# revision 55
# speedup vs baseline: 1.0439x; 1.0439x over previous
"""Causal self-attention (GPT-2 small block shape: B=4, T=2048, C=768, H=12, D=64)
on 8 TRN2 NeuronCores.

Sharding: core i handles batch b = i//2 and head-half = i%2 (6 heads each).
No cross-core collectives; the two half-head partial output projections per
batch are summed on the host during unshard (row-parallel c_proj).

Optimizations over the first working version (228us -> 206us):
  - S matmuls for the two heads of a pair are issued back-to-back as
    64-contraction row-tiles (rows 0:64 / 64:128 of the PE array) so they
    stream CONCURRENTLY -> S-gen PE cost halves.
  - One ScalarE exp per k-chunk covers both heads ([128,1024] f32 PSUM
    tile), with a strided AP that skips the causally-dead leading columns
    of diagonal chunks; exp table preloaded at kernel start.
  - Global software pipeline: AV of chunk j is emitted during chunk j+1;
    QKV/V/proj matmuls are scheduled as PE fill into the ACT-bound late
    rounds; av-bank evacuation is decoupled from the reciprocal/broadcast/
    multiply normalization tail (which drains one-per-chunk in later slack)
    so the next head-pair's AV is never WAR-blocked on the norm chain.
  - V/proj biases folded in as K=1 ones-outer-product matmuls; reciprocal
    runs on a partition-0 copy (the custom DVE recip uop cannot read
    across partitions - plain COPY can).
  - Input DMAs split per-use across sync/scalar/gpsimd queues in
    first-use order (first matmul at ~13us vs ~23us); output stored as
    bf16 (host upcasts + sums); keep-warm matmuls bridge the tail norm
    chain so the final projections don't run HAM-throttled.
"""

import sys

if "/opt/trn_rl_repo" not in sys.path:
    sys.path.insert(0, "/opt/trn_rl_repo")

import numpy as np
import ml_dtypes

import concourse.bass as bass  # noqa: F401
import concourse.mybir as mybir
from concourse import bacc
from concourse.tile import TileContext
from concourse.bass_utils import run_bass_kernel_spmd

BF16 = ml_dtypes.bfloat16

B, T, C = 4, 2048, 768
H, D = 12, 64
NH = 6  # heads per core
P = 128
TC = T // P  # 16 t-chunks of 128
QC = T // 512  # 4 q-blocks of 512
CCH = C // P  # 6 contraction chunks

DT = mybir.dt.bfloat16
F32 = mybir.dt.float32


def build_nc():
    nc = bacc.Bacc()

    xt_d = nc.declare_dram_parameter("xt", [P, T // 512, CCH, 512], DT, isOutput=False)
    # fc-major so per-fc chunks are a single contiguous DMA
    wqk_d = nc.declare_dram_parameter("wqk", [P, 2 * NH * D // P, CCH, P], DT, isOutput=False)
    bqk_d = nc.declare_dram_parameter("bqk", [P, 2 * NH * D // P], F32, isOutput=False)
    wv_d = nc.declare_dram_parameter("wv", [P, CCH, NH * D], DT, isOutput=False)
    bv_d = nc.declare_dram_parameter("bv", [1, NH * D], DT, isOutput=False)
    wp_d = nc.declare_dram_parameter("wp", [P, NH * D // P, C], DT, isOutput=False)
    bp_d = nc.declare_dram_parameter("bp", [1, C], DT, isOutput=False)
    mask_d = nc.declare_dram_parameter("mask", [P, 2 * P], DT, isOutput=False)
    out_d = nc.declare_dram_parameter("out", [T, C], DT, isOutput=True)

    NFC = 2 * NH * D // P  # 6 q/k feature chunks

    with TileContext(nc) as tc:
        with (
            tc.tile_pool(name="consts", bufs=1) as consts,
            tc.tile_pool(name="work", bufs=3) as work,
            tc.tile_pool(name="outp", bufs=3) as outp,
            tc.tile_pool(name="ps_s", bufs=2, space="PSUM") as ps_s,
            tc.tile_pool(name="ps_qkv", bufs=2, space="PSUM") as ps_qkv,
            tc.tile_pool(name="ps_av", bufs=2, space="PSUM") as ps_av,
        ):
            # ---- input DMAs, ordered by first use ----
            # xt is quarter-major [P, 4, CCH, 512]; each quarter's halves
            # (cc 0:3 / 3:6) stream on sync/scalar in parallel.
            bqk_sb = consts.tile([P, NFC], F32)
            nc.gpsimd.dma_start(bqk_sb[:], bqk_d[:])
            xt_sb = consts.tile([P, QC, CCH, 512], DT)
            wqk_sb = consts.tile([P, NFC, CCH, P], DT)
            nc.sync.dma_start(xt_sb[:, 0, 0:3], xt_d[:, 0, 0:3])
            nc.scalar.dma_start(xt_sb[:, 0, 3:6], xt_d[:, 0, 3:6])
            nc.sync.dma_start(wqk_sb[:, 0], wqk_d[:, 0])
            nc.scalar.dma_start(wqk_sb[:, 3], wqk_d[:, 3])
            nc.sync.dma_start(wqk_sb[:, 1], wqk_d[:, 1])
            nc.scalar.dma_start(wqk_sb[:, 4], wqk_d[:, 4])
            nc.sync.dma_start(wqk_sb[:, 2], wqk_d[:, 2])
            nc.scalar.dma_start(wqk_sb[:, 5], wqk_d[:, 5])
            mask2_sb = consts.tile([P, 2 * P], DT)
            nc.gpsimd.dma_start(mask2_sb[:], mask_d[:])
            bv_sb = consts.tile([1, NH * D], DT)
            nc.gpsimd.dma_start(bv_sb[:], bv_d[:])
            wv_sb = consts.tile([P, CCH, NH * D], DT)
            nc.gpsimd.dma_start(wv_sb[:], wv_d[:])
            for q in range(1, QC):
                nc.sync.dma_start(xt_sb[:, q, 0:3], xt_d[:, q, 0:3])
                nc.scalar.dma_start(xt_sb[:, q, 3:6], xt_d[:, q, 3:6])
            bp_sb = consts.tile([1, C], DT)
            nc.gpsimd.dma_start(bp_sb[:], bp_d[:])
            wp_sb = consts.tile([P, NH * D // P, C], DT)
            nc.gpsimd.dma_start(wp_sb[:], wp_d[:])

            # ones row for K=1 bias-broadcast matmuls (no DMA dependency)
            ones_row = consts.tile([1, P], DT)
            nc.gpsimd.memset(ones_row[:], 1.0)
            bpb = consts.tile([P, C], DT)
            nc.gpsimd.partition_broadcast(bpb[:], bp_sb[:])
            # preload the exp table set while input DMAs stream
            warm = consts.tile([1, 16], F32)
            nc.gpsimd.memset(warm[:], 0.0)
            warm2 = consts.tile([1, 16], DT)
            nc.scalar.activation(warm2[:], warm[:], mybir.ActivationFunctionType.Exp)

            # Q^T/K^T head-pair tiles [128, T]: head 2p in partitions 0:64,
            # head 2p+1 in partitions 64:128
            qtp = [consts.tile([P, T], DT, name=f"qtp{p}", tag=f"qtp{p}") for p in range(NH // 2)]
            ktp = [consts.tile([P, T], DT, name=f"ktp{p}", tag=f"ktp{p}") for p in range(NH // 2)]
            # V per t-chunk, heads side by side with a ones column: [128, 6, 65]
            vt = [consts.tile([P, NH, D + 1], DT, name=f"vt{t}", tag=f"vt{t}") for t in range(TC)]
            for t in range(TC):
                nc.gpsimd.memset(vt[t][:, :, D : D + 1], 1.0)
            # y^T per head-pair [128, T] bf16
            yt = [consts.tile([P, T], DT, name=f"yt{p}", tag=f"yt{p}") for p in range(NH // 2)]

            # ---- QKV / proj helpers ----
            def qk_tile(fc, qcb):
                pq = ps_qkv.tile([P, 512], F32, tag="qkv", name="pq")
                for cc in range(CCH):
                    nc.tensor.matmul(
                        pq[:],
                        wqk_sb[:, fc, cc, :],
                        xt_sb[:, qcb, cc, :],
                        start=(cc == 0),
                        stop=(cc == CCH - 1),
                    )
                dst = qtp[fc] if fc < 3 else ktp[fc - 3]
                nc.vector.tensor_scalar_add(
                    dst[:, qcb * 512 : (qcb + 1) * 512],
                    pq[:],
                    bqk_sb[:, fc : fc + 1],
                )

            def v_chunk(t):
                pv = ps_qkv.tile([P, NH * D], F32, tag="qkv", name="pv")
                for cc in range(CCH):
                    nc.tensor.matmul(
                        pv[:],
                        xt_sb[:, t // 4, cc, (t % 4) * P : (t % 4 + 1) * P],
                        wv_sb[:, cc, :],
                        start=(cc == 0),
                        stop=False,
                    )
                # bias via K=1 outer product: ones[1,128].T @ bv[1,384]
                nc.tensor.matmul(
                    pv[:], ones_row[:], bv_sb[:], start=False, stop=True
                )
                nc.vector.tensor_copy(
                    vt[t][:, :, 0:D], pv[:].rearrange("p (h d) -> p h d", d=D)
                )

            def proj(t, pool=None, fold_bias=False):
                pool = pool or ps_qkv
                tg = "qkv" if pool is ps_qkv else "av"
                ppa = pool.tile([P, 512], F32, tag=tg, name="ppa")
                ppb = pool.tile([P, C - 512], F32, tag=tg, name="ppb")
                for pp, c0, c1 in ((ppa, 0, 512), (ppb, 512, C)):
                    for cp in range(NH * D // P):
                        nc.tensor.matmul(
                            pp[:, 0 : c1 - c0],
                            yt[cp][:, t * P : (t + 1) * P],
                            wp_sb[:, cp, c0:c1],
                            start=(cp == 0),
                            stop=(cp == NH * D // P - 1) and not fold_bias,
                        )
                    if fold_bias:
                        nc.tensor.matmul(
                            pp[:, 0 : c1 - c0],
                            ones_row[:],
                            bp_sb[:, c0:c1],
                            start=False,
                            stop=True,
                        )
                stg = outp.tile([P, C], DT)
                if fold_bias:
                    nc.vector.tensor_copy(stg[:, 0:512], ppa[:])
                    nc.vector.tensor_copy(stg[:, 512:C], ppb[:])
                else:
                    nc.vector.tensor_add(stg[:, 0:512], ppa[:], bpb[:, 0:512])
                    nc.vector.tensor_add(stg[:, 512:C], ppb[:], bpb[:, 512:C])
                nc.sync.dma_start(out_d[t * P : (t + 1) * P, :], stg[:])

            # ---- attention pieces ----
            # per (hp, qc): loop over groups g of 2 k-chunks; S for both heads
            # of the pair as concurrent row-tiles into one bf16 psum tile;
            # one exp per group; AV emitted one group behind (pipelined).

            av_tiles = {}

            def emit_s_group(hp, qc, j):
                # S for both heads of the pair as concurrent 64-row tiles:
                # head 0 -> [0:512] (bank a), head 1 -> [512:1024] (bank b)
                m = max(0, (j - 4 * qc) * P)
                sps = ps_s.tile([P, 1024], F32, tag="s", name="sps")
                for hi in (0, 1):
                    b0 = 64 * hi
                    nc.tensor.matmul(
                        sps[:, hi * 512 + m : (hi + 1) * 512],
                        ktp[hp][b0 : b0 + 64, j * P : (j + 1) * P],
                        qtp[hp][b0 : b0 + 64, qc * 512 + m : (qc + 1) * 512],
                        start=True,
                        stop=True,
                    )
                sexp = work.tile([P, 1024], DT, tag="sexp", bufs=6, name="sexp")
                if m:
                    # skip the causally-dead leading m columns of each half
                    nc.scalar.activation(
                        sexp[:].rearrange("p (h q) -> p h q", h=2)[:, :, m:],
                        sps[:].rearrange("p (h q) -> p h q", h=2)[:, :, m:],
                        mybir.ActivationFunctionType.Exp,
                    )
                else:
                    nc.scalar.activation(
                        sexp[:], sps[:], mybir.ActivationFunctionType.Exp
                    )
                if j - 4 * qc >= 0:  # diagonal block: mask upper triangle
                    # one strided TT covers both heads' diagonal squares
                    view = sexp[:].rearrange("p (h q) -> p h q", h=2)[:, :, m : m + P]
                    nc.vector.tensor_mul(
                        view, view, mask2_sb[:].rearrange("p (h q) -> p h q", h=2)
                    )
                return sexp

            def emit_av_group(hp, qc, j, sexp):
                nj = 4 * (qc + 1)
                m = max(0, (j - 4 * qc) * P)
                for hi in (0, 1):
                    av = av_tiles[(hp, qc, hi)]
                    nc.tensor.matmul(
                        av[:, m:512],
                        vt[j][:, 2 * hp + hi, :],
                        sexp[:, hi * 512 + m : (hi + 1) * 512],
                        start=(j == 0),
                        stop=(j == nj - 1),
                        skip_group_check=True,
                    )

            # Normalization is split: a fast PSUM->SBUF evacuation (frees the
            # av bank ~2us earlier so the next head-pair's AV isn't WAR-
            # blocked), then a deferred recip/broadcast/multiply chain that
            # runs in later slack.
            def emit_evac(hp, qc, hi):
                av = av_tiles.pop((hp, qc, hi))
                avf = work.tile([65, 512], F32, tag="avf", bufs=6, name="avf")
                nc.vector.tensor_copy(avf[:], av[:])
                return avf

            last_inv = [None]

            def emit_norm_tail(hp, qc, hi, avf):
                b0 = 64 * hi
                # partition-shift copy (64 -> 0): reciprocal's custom uop
                # cannot read across partitions, plain COPY can
                sums = work.tile([1, 512], F32, tag="sums", name="sums")
                nc.vector.tensor_copy(sums[:], avf[64:65, :])
                inv = work.tile([1, 512], F32, tag="inv", name="inv")
                nc.vector.reciprocal_approx_fast(inv[:], sums[:])
                invb = work.tile([64, 512], F32, tag="invb", name="invb")
                nc.gpsimd.partition_broadcast(invb[:], inv[:])
                nc.vector.tensor_mul(
                    yt[hp][b0 : b0 + 64, qc * 512 : (qc + 1) * 512],
                    avf[0:64, :],
                    invb[:],
                )
                last_inv[0] = inv

            # ---- prologue: round-0 QKV ----
            for fc in (0, 3, 1, 4, 2, 5):
                qk_tile(fc, 0)
            for t in range(4):
                v_chunk(t)

            # ---- global pipeline over rounds ----
            # Per head-pair: AV of group g is emitted during group g+1 so the
            # ScalarE exp hides behind PE work; the last AV + normalization
            # are emitted before the next head-pair's masks enter the DVE
            # queue (avoids head-of-line blocking of the norm chain).
            deferred = []  # (hp, qc, hi, ytu, sums) norm tails awaiting slack

            def drain_norms(limit=None):
                n = 0
                while deferred and (limit is None or n < limit):
                    emit_norm_tail(*deferred.pop(0))
                    n += 1

            # proj(t) is scheduled late so the ACT-bound rounds 2-3 get the
            # most PE fill work (round 1 is PE-bound: no projs there);
            # round 3 holds 3 projs back for the tail.
            proj_rounds = {2: [2, 3, 4, 5], 3: [0, 1, 6, 7, 8, 9, 10, 11]}
            for qc in range(QC):
                nj = 4 * (qc + 1)
                fill = []
                if qc < QC - 1:
                    for fc in (0, 3, 1, 4, 2, 5):
                        fill.append(lambda fc=fc: qk_tile(fc, qc + 1))
                    for t in range(4 * (qc + 1), 4 * (qc + 2)):
                        fill.append(lambda t=t: v_chunk(t))
                for t in proj_rounds.get(qc, []):
                    fill.append(lambda t=t: proj(t))
                reserve = 3 if qc == QC - 1 else 0
                pace = (len(fill) - reserve) / (3 * nj)
                acc = 0.0
                fi = 0
                for hp in range(3):
                    for hi in (0, 1):
                        av_tiles[(hp, qc, hi)] = ps_av.tile(
                            [65, 512], F32, tag="av", name="av"
                        )
                    prev = None
                    for j in range(nj):
                        sexp = emit_s_group(hp, qc, j)
                        if prev is not None:
                            emit_av_group(hp, qc, j - 1, prev)
                            drain_norms(limit=1)
                        prev = sexp
                        acc += pace
                        while acc >= 1.0 and fi < len(fill):
                            fill[fi]()
                            fi += 1
                            acc -= 1.0
                    if fi < len(fill) - reserve:  # cover this pair's final exp
                        fill[fi]()
                        fi += 1
                        acc -= 1.0
                    emit_av_group(hp, qc, nj - 1, prev)
                    for hi in (0, 1):
                        avf = emit_evac(hp, qc, hi)
                        deferred.append((hp, qc, hi, avf))
                if qc == QC - 1:
                    # start the final norm chains, then cover them with the
                    # reserved projs + keep-warm matmuls so the PE doesn't
                    # HAM-throttle before the last projections
                    drain_norms()
                while fi < len(fill):
                    fill[fi]()
                    fi += 1
            for w in range(16):
                wps = ps_s.tile([P, 1024], F32, tag="s", name="wps")
                nc.tensor.matmul(
                    wps[:, 0:512], ones_row[:], bp_sb[:, 0:512], start=True, stop=True
                )
            for t in range(4 * (QC - 1), 4 * QC):
                proj(t, pool=(ps_qkv if t % 2 == 0 else ps_av), fold_bias=True)

    nc.finalize()
    return nc


def shard_inputs(x, w_attn, b_attn, w_proj, b_proj):
    """Host-side prep: slice per core, transpose x, cast to bf16."""
    scale = 1.0 / np.sqrt(D)
    tril = np.tril(np.ones((P, P), np.float32))
    # mask[k_local, q_local] = 1 where k <= q; duplicated for the two heads
    mask = np.concatenate([tril.T, tril.T], axis=1).astype(BF16)
    NFC = 2 * NH * D // P
    in_maps = []
    for core in range(8):
        b, half = divmod(core, 2)
        h0 = half * NH
        cq = slice(h0 * D, (h0 + NH) * D)
        ck = slice(C + h0 * D, C + (h0 + NH) * D)
        cv = slice(2 * C + h0 * D, 2 * C + (h0 + NH) * D)
        wq = (w_attn[:, cq] * scale).astype(BF16)
        wk = w_attn[:, ck].astype(BF16)
        wqk = np.concatenate([wq, wk], axis=1)  # [C, 768]
        bqk = np.concatenate([(b_attn[cq] * scale), b_attn[ck]], axis=0).astype(
            np.float32
        )
        bqk_col = np.ascontiguousarray(bqk.reshape(NFC, P).T)
        wv = w_attn[:, cv].astype(BF16)
        bv = b_attn[cv].astype(BF16)[None, :]
        wp = w_proj[h0 * D : (h0 + NH) * D, :].astype(BF16)
        bp = (b_proj if half == 0 else np.zeros_like(b_proj)).astype(BF16)[None, :]
        xt = np.ascontiguousarray(x[b].T).astype(BF16)  # [C, T]
        # wqk fc-major: [P, fc, cc, 128] with wqk[cc*128+p, fc*128+k]
        wqk_dev = np.ascontiguousarray(
            wqk.reshape(CCH, P, NFC, P).transpose(1, 2, 0, 3)
        )
        in_maps.append(
            {
                # quarter-major: [P, q, cc, 512]
                "xt": np.ascontiguousarray(
                    xt.reshape(CCH, P, T // 512, 512).transpose(1, 2, 0, 3)
                ),
                "wqk": wqk_dev,
                "bqk": bqk_col,
                "wv": np.ascontiguousarray(
                    wv.reshape(CCH, P, NH * D).transpose(1, 0, 2)
                ),
                "bv": bv,
                "wp": np.ascontiguousarray(
                    wp.reshape(NH * D // P, P, C).transpose(1, 0, 2)
                ),
                "bp": bp,
                "mask": mask,
            }
        )
    return in_maps


_NC = None


def _get_nc():
    global _NC
    if _NC is None:
        _NC = build_nc()
    return _NC


def run_sharded(in_maps, trace=False, **kw):
    nc = _get_nc()
    return run_bass_kernel_spmd(nc, in_maps, core_ids=list(range(8)), trace=trace, **kw)


def gather(results):
    out = np.zeros((B, T, C), np.float32)
    for core in range(8):
        b = core // 2
        out[b] += results[core]["out"].astype(np.float32)
    return out


def kernel(x, w_attn, b_attn, w_proj, b_proj):
    x = np.asarray(x, np.float32)
    w_attn = np.asarray(w_attn, np.float32)
    b_attn = np.asarray(b_attn, np.float32)
    w_proj = np.asarray(w_proj, np.float32)
    b_proj = np.asarray(b_proj, np.float32)
    in_maps = shard_inputs(x, w_attn, b_attn, w_proj, b_proj)
    res = run_sharded(in_maps, trace=False)
    return gather(res.results)
